# revision 12
# baseline (speedup 1.0000x reference)
"""Causal self-attention (B=2, T=2048, C=768, H=12) on 8 TRN2 NeuronCores.

Sharding: 24 (batch, head) pairs -> 8 cores x 3 heads (head-tensor-parallel
within a batch, data-parallel across the 2 batches: cores 0-3 = batch 0,
cores 4-7 = batch 1). Each core computes qkv for its 3 heads, causal
attention, and a rank-192 partial of the output projection; the host sums
the 4 partials per batch and adds b_proj.

Per-core kernel:
  - qkv matmuls run in fp8e4m3 DoubleRow mode (2 contraction k-tiles per
    matmul, 2x PE rate). W_attn is pre-scaled by 16 on the host to center
    its values in e4m3 range; the 16x on q/k folds into the exp scale
    (0.125/256) and the 16x on v folds into W_proj (/16).
  - everything else (scores, O, proj) in bf16: fp32r runs 4x slow on this
    hardware for dense matmul streams; bf16 is 1 cyc/row unconditionally.
  - scores computed transposed: S^T[k, q] = k . q  per 128-row k-tile, so
    softmax needs no max subtraction and O^T accumulates over k-tiles in
    PSUM. The softmax denominator falls out of the same matmul via a
    ones-column appended to the v stationary (65th output row).
  - attention iterates q-chunk-outer / k-tile-inner, two k-tiles paired per
    PSUM tile so each exp is one big ACT op; ACT does (almost) nothing but
    exp — qkv bias copies go to DVE, v-transpose copies split DVE/Pool.
  - causal mask: gpsimd zeroes the invalid triangle of exp(S) on diagonal
    tiles only; fully masked tiles are skipped.
  - the timing loop (loop_n>1) is unrolled 2x with ping-pong buffer slots
    so iteration i+1's input DMAs + qkv overlap iteration i's attention
    (without this, phases serialize across iterations on the WAR hazards
    of single-buffered persistent tiles).
"""

import os
import sys

for _p in ("/opt/trn_rl_repo", "/root/.axon_site/_ro/trn_rl_repo"):
    if os.path.isdir(_p) and _p not in sys.path:
        sys.path.insert(0, _p)

import numpy as np

import concourse.bass as bass  # noqa: F401
import concourse.mybir as mybir
import concourse.tile as tile
from concourse import bacc
from concourse.bass_utils import run_bass_kernel_spmd
from concourse.masks import make_identity

B, T, C, H, DH = 2, 2048, 768, 12, 64
HPC = 3          # heads per core
NCORES = 8
KO = C // 128    # 6 contraction tiles over the model dim
F32 = mybir.dt.float32
BF16 = mybir.dt.bfloat16
FP8 = mybir.dt.float8e4
AF = mybir.ActivationFunctionType
ALU = mybir.AluOpType
DR = mybir.MatmulPerfMode.DoubleRow
QKV_FP8 = False  # fp8 DoubleRow qkv: q/k only (v stays bf16; see simcheck)
W_SCALE = 16.0 if QKV_FP8 else 1.0  # host pre-scale of W_attn into e4m3 range
EXP_SCALE = 0.125 / (W_SCALE * W_SCALE)
E_BUFS, N_BUFS, Y_BUFS, S_BUFS, O_BUFS = 6, 4, 4, 3, 2
WARMUP_MM = 24

# qkv row groups: G0=[q0|q1] G1=[k0|k1] G2=[v0|v1] G3=[q2|v2] G4=[k2|pad]
# (matmul requires lhsT/rhs at the same partition base, so each head's q and
# k must share a base: h0/h2 at base 0, h1 at base 64)
Q_POS = {0: (0, 0), 1: (0, 64), 2: (3, 0)}
K_POS = {0: (1, 0), 1: (1, 64), 2: (4, 0)}
V_POS = {0: (2, 0), 1: (2, 64), 2: (3, 64)}
# host-side column order matching the groups ((kind 0=q/1=k/2=v, head idx))
W_ORDER = [(0, 0), (0, 1), (1, 0), (1, 1), (2, 0), (2, 1), (0, 2), (2, 2),
           (1, 2)]


from ml_dtypes import bfloat16 as np_bf16
from ml_dtypes import float8_e4m3 as np_fp8


def to_bf16(a):
    return np.ascontiguousarray(a, np.float32).astype(np_bf16)


def to_fp8(a):
    return np.ascontiguousarray(a, np.float32).astype(np_fp8)


def _alloc_slot(pp, s):
    """Per-slot persistent tiles; two slots ping-pong across the timing
    loop so consecutive iterations have no WAR hazards on these."""
    t = {}
    t["b_sb"] = pp.tile([128, 5], F32, name=f"b_sb{s}")
    t["ones_col"] = pp.tile([128, 1], BF16, name=f"ones{s}")
    wdt = FP8 if QKV_FP8 else BF16
    t["w_sb"] = pp.tile([128, KO, 576], wdt, name=f"w_sb{s}")
    t["xt_sb"] = pp.tile([128, KO, T], wdt, name=f"xt_sb{s}")
    t["wpa"] = pp.tile([128, C], BF16, name=f"wpa{s}")
    t["wpb"] = pp.tile([64, C], BF16, name=f"wpb{s}")
    t["qkvT"] = pp.tile([128, 5, T], BF16, name=f"qkvT{s}")
    t["v_sb"] = [pp.tile([128, 16, 65], BF16, name=f"v_sb{s}_{h}")
                 for h in range(HPC)]
    if QKV_FP8:
        t["xv0_sb"] = pp.tile([128, KO, 128], BF16, name=f"xv0{s}")
        t["wv_sb"] = pp.tile([128, KO, 192], BF16, name=f"wv{s}")
        t["wqk0_sb"] = pp.tile([128, KO, 384], BF16, name=f"wqk0{s}")
        t["vT0"] = pp.tile([128, 2, 128], BF16, name=f"vT0{s}")
    t["OT_a"] = pp.tile([128, T], BF16, name=f"OT_a{s}")
    t["OT_b"] = pp.tile([64, T], BF16, name=f"OT_b{s}")
    t["h1tmp"] = pp.tile([64, T], BF16, name=f"h1tmp{s}")
    return t


def _build_body(nc, tc, pools, ident, S, xt_d, w_d, b_d, wp_d, y_d,
                xv0_d=None, wv_d=None, wqk0_d=None,
                phases=('qkv', 'attn', 'proj'), warmup=0):
    pp, sbE, sbN, sbY, psp = pools
    b_sb, ones_col = S["b_sb"], S["ones_col"]
    w_sb, xt_sb = S["w_sb"], S["xt_sb"]
    wpa, wpb, qkvT, v_sb = S["wpa"], S["wpb"], S["qkvT"], S["v_sb"]
    OT_a, OT_b, h1tmp = S["OT_a"], S["OT_b"], S["h1tmp"]

    if warmup:
        # keep the HAM activity monitor busy while the first input DMAs
        # land so real matmuls start at speed (prologue body only)
        warm = psp.tile([128, 1024], F32, name="warm", tag="S", bufs=S_BUFS)
        for wi in range(warmup):
            nc.tensor.matmul(warm[:, 0:128], ident, ident,
                             start=True, stop=True, skip_group_check=True)

    # -- input DMAs (bias+weights first — the first qkv matmul needs
    # w_sb and xt chunk 0; wp last: only needed by the projection) ----
    nc.sync.dma_start(b_sb[:], b_d[:])
    nc.scalar.activation(ones_col, b_sb[:, 0:1], AF.Copy, bias=1.0, scale=0.0)
    nc.sync.dma_start(w_sb, w_d.rearrange("(ko p) m -> p ko m", p=128))
    xt_r = xt_d.rearrange("(ko p) n -> p ko n", p=128)
    for nch in range(4):
        nc.sync.dma_start(
            xt_sb[:, :, 512 * nch:512 * (nch + 1)],
            xt_r[:, :, 512 * nch:512 * (nch + 1)],
        )
    if QKV_FP8:
        nc.sync.dma_start(S["xv0_sb"],
                          xv0_d.rearrange("(ko p) n -> p ko n", p=128))
        nc.sync.dma_start(S["wv_sb"],
                          wv_d.rearrange("(ko p) m -> p ko m", p=128))
        nc.sync.dma_start(S["wqk0_sb"],
                          wqk0_d.rearrange("(ko p) m -> p ko m", p=128))
    nc.sync.dma_start(wpa, wp_d[0:128, :])
    nc.sync.dma_start(wpb, wp_d[128:192, :])

    def s_tile(name):
        return psp.tile([128, 1024], F32, name=name, tag="S", bufs=S_BUFS)

    # -- phase 1: qkv^T = W_slice^T @ x^T in fp8 DoubleRow (g-outer so each
    #    head's q/k/v complete as early as possible; v transposed JIT)
    # bf16 recompute of v^T[:, 0:128]: V0_POS[h] = (group in vT0, row base)
    V0_POS = {0: (0, 0), 1: (0, 64), 2: (1, 64)}

    def v0_group(gg):
        # gg=0: [v0|v1] rows 0..127 (b_sb col 2); gg=1: v2 rows 64..127
        # (b_sb col 3 = bias of qkv group 3 whose rows 64.. are v2)
        vT0 = S["vT0"]
        r0, M, bcol = (0, 128, 2) if gg == 0 else (64, 64, 3)
        pv = psp.tile([128, 128], F32, name="pv", tag="S", bufs=S_BUFS)
        for ko in range(KO):
            nc.tensor.matmul(
                pv[r0:r0 + M, 0:128],
                S["wv_sb"][:, ko, 64 * gg * 2:64 * gg * 2 + M],
                S["xv0_sb"][:, ko, :],
                start=(ko == 0), stop=(ko == KO - 1),
            )
        nc.vector.tensor_scalar_add(vT0[r0:r0 + M, gg, :],
                                    pv[r0:r0 + M, 0:128],
                                    b_sb[r0:r0 + M, bcol:bcol + 1])

    # wqk0 column blocks: (qkv group, rows M, col offset in wqk0)
    QK0_BLOCKS = {0: (0, 128, 0), 1: (1, 128, 128),
                  3: (3, 64, 256), 4: (4, 64, 320)}

    def qk0_fix(g):
        _, M, off = QK0_BLOCKS[g]
        pq = psp.tile([128, 128], F32, name="pq", tag="S", bufs=S_BUFS)
        for ko in range(KO):
            nc.tensor.matmul(
                pq[:M, 0:128],
                S["wqk0_sb"][:, ko, off:off + M],
                S["xv0_sb"][:, ko, :],
                start=(ko == 0), stop=(ko == KO - 1),
            )
        nc.vector.tensor_scalar_add(qkvT[:M, g, 0:128], pq[:M, 0:128],
                                     b_sb[:M, g:g + 1])

    def transpose_v(h):
        nc.vector.tensor_copy(v_sb[h][:, :, 64:65],
                              ones_col.broadcast_to([128, 16, 1]))
        g, r0 = V_POS[h]
        vT = qkvT[r0:r0 + 64, g, :]
        for kt in range(16):
            tp = psp.tile([128, 64], BF16, name="tp_ps", tag="S",
                          bufs=S_BUFS)
            if QKV_FP8 and kt == 0:
                gv, r0v = V0_POS[h]
                src_ap = S["vT0"][r0v:r0v + 64, gv, :]
                idn = ident[r0v:r0v + 64, r0v:r0v + 64]
            else:
                src_ap = vT[:, 128 * kt:128 * (kt + 1)]
                idn = ident[r0:r0 + 64, r0:r0 + 64]
            nc.tensor.transpose(tp[:, 0:64], src_ap, idn)
            if kt % 2 == 0:
                nc.vector.tensor_copy(v_sb[h][:, kt, 0:64], tp[:, 0:64])
            else:
                nc.scalar.copy(v_sb[h][:, kt, 0:64], tp[:, 0:64])

    def qkv_group(g):
        M = 128 if g < 4 else 64
        for nch in range(4):
            ps = s_tile("qkv_ps")
            if QKV_FP8:
                for kk in range(KO // 2):
                    nc.tensor.matmul(
                        ps[:M, 0:512],
                        w_sb[:, 2 * kk:2 * kk + 2, g * 128:g * 128 + M],
                        xt_sb[:, 2 * kk:2 * kk + 2, 512 * nch:512 * (nch + 1)],
                        start=(kk == 0), stop=(kk == KO // 2 - 1),
                        perf_mode=DR,
                    )
            else:
                for ko in range(KO):
                    nc.tensor.matmul(
                        ps[:M, 0:512],
                        w_sb[:, ko, g * 128:g * 128 + M],
                        xt_sb[:, ko, 512 * nch:512 * (nch + 1)],
                        start=(ko == 0), stop=(ko == KO - 1),
                    )
            dst = qkvT[:M, g, 512 * nch:512 * (nch + 1)]
            nc.vector.tensor_scalar_add(dst, ps[:M, 0:512], b_sb[:M, g:g + 1])

    if 'qkv' in phases:
        for g in (0, 1, 2):
            qkv_group(g)
        if QKV_FP8:
            v0_group(0)
            qk0_fix(0)
            qk0_fix(1)
        transpose_v(0)
        transpose_v(1)

    # -- phase 2+3: attention chunk-outer across heads, projection of
    #    each 512-wide q-chunk as soon as all three heads finish it ---
    def attn_chunk(h, j):
        qg, qb = Q_POS[h]
        kg, kb = K_POS[h]
        qT = qkvT[qb:qb + 64, qg, :]
        kT = qkvT[kb:kb + 64, kg, :]
        O_t = psp.tile([65, 512], F32, name=f"O_{h}_{j}", tag="O", bufs=O_BUFS)
        n_i = 4 * j + 4          # k-tiles contributing to this chunk
        for ip in range(0, n_i, 2):   # pairs (ip, ip+1)
            sp = s_tile(f"s_{h}_{j}_{ip}")
            E = sbE.tile([128, 1024], BF16, name="E", tag="E")
            chunks = []
            off = 0
            for i in (ip, ip + 1):
                cs = max(128 * i, 512 * j)
                ce = 512 * (j + 1)
                w = ce - cs
                nc.tensor.matmul(
                    sp[:, off:off + w],
                    kT[:, 128 * i:128 * (i + 1)],
                    qT[:, cs:ce],
                    start=True, stop=True,
                )
                chunks.append((i, cs, off, w))
                # keep each matmul inside one PSUM bank: full 512 ->
                # bank 1 (off 512), partials pack back-to-back in bank 0
                off = 512 if w == 512 else off + w
            total = chunks[-1][2] + chunks[-1][3]
            nc.scalar.activation(E[:, 0:total], sp[:, 0:total],
                                 AF.Exp, scale=EXP_SCALE)
            for i, cs, off_i, w in chunks:
                if cs == 128 * i:
                    # diagonal tile: zero E where k > q (strictly lower
                    # triangle of the 128-wide diagonal block)
                    nc.gpsimd.affine_select(
                        out=E[:, off_i:off_i + 128],
                        in_=E[:, off_i:off_i + 128],
                        compare_op=ALU.is_ge, fill=0.0,
                        base=0, pattern=[[1, 128]], channel_multiplier=-1,
                    )
            for i, cs, off_i, w in chunks:
                nc.tensor.matmul(
                    O_t[:, cs - 512 * j:cs - 512 * j + w],
                    v_sb[h][:, i, :],
                    E[:, off_i:off_i + w],
                    start=(i == 0), stop=(i == n_i - 1),
                )
            yield
        # normalize O^T rows 0..63 by row 64 (the exp-sum)
        recip = sbN.tile([1, 512], F32, name="recip", tag="recip")
        nc.vector.reciprocal(recip, O_t[64:65, :])
        bc = sbN.tile([64, 512], F32, name="bc", tag="bc")
        nc.gpsimd.partition_broadcast(bc, recip, channels=64)
        if h == 0:
            dst = OT_a[0:64, 512 * j:512 * (j + 1)]
        elif h == 1:
            dst = h1tmp[:, 512 * j:512 * (j + 1)]
        else:
            dst = OT_b[:, 512 * j:512 * (j + 1)]
        nc.vector.tensor_tensor(dst, O_t[0:64, :], bc, ALU.mult)
        if h == 1:
            # head 1 lives on partitions 64..127 of the proj stationary
            nc.sync.dma_start(OT_a[64:128, 512 * j:512 * (j + 1)],
                              h1tmp[:, 512 * j:512 * (j + 1)])

    def proj_tile(m):
        qsl = slice(128 * m, 128 * (m + 1))
        ya = s_tile("ya")
        yb = s_tile("yb")
        nc.tensor.matmul(ya[:, 0:512], OT_a[:, qsl], wpa[:, 0:512],
                         start=True, stop=False)
        nc.tensor.matmul(ya[:, 0:512], OT_b[:, qsl], wpb[:, 0:512],
                         start=False, stop=True)
        nc.tensor.matmul(yb[:, 0:256], OT_a[:, qsl], wpa[:, 512:768],
                         start=True, stop=False)
        nc.tensor.matmul(yb[:, 0:256], OT_b[:, qsl], wpb[:, 512:768],
                         start=False, stop=True)
        ysb = sbY.tile([128, C], F32, name="ysb", tag="ysb")
        nc.scalar.copy(ysb[:, 0:512], ya[:, 0:512])
        nc.vector.tensor_copy(ysb[:, 512:768], yb[:, 0:256])
        nc.sync.dma_start(y_d[qsl, :], ysb)

    def interleave(gens):
        live = list(gens)
        while live:
            nxt = []
            for g in live:
                try:
                    next(g)
                    nxt.append(g)
                except StopIteration:
                    pass
            live = nxt

    if 'attn' in phases:
        for j in range(4):
            interleave([attn_chunk(0, j), attn_chunk(1, j)])
        if 'qkv' in phases:
            qkv_group(3)
            qkv_group(4)
            if QKV_FP8:
                v0_group(1)
                qk0_fix(3)
                qk0_fix(4)
            transpose_v(2)
        for j in range(4):
            interleave([attn_chunk(2, j)])
            if 'proj' in phases and j > 0:
                for m in range(4 * (j - 1), 4 * j):
                    proj_tile(m)
        if 'proj' in phases:
            for m in range(12, 16):
                proj_tile(m)
    elif 'qkv' in phases:
        qkv_group(3)
        qkv_group(4)
        if QKV_FP8:
            v0_group(1)
            qk0_fix(3)
            qk0_fix(4)
        transpose_v(2)
    if 'proj' not in phases or 'attn' not in phases:
        # stand-in output writeback so every variant writes y identically
        for m in range(16):
            ysb = sbY.tile([128, C], F32, name="ysb", tag="ysb")
            nc.vector.memset(ysb, 0.0)
            nc.sync.dma_start(y_d[128 * m:128 * (m + 1), :], ysb)


def build_module(loop_n=1, phases=('qkv', 'attn', 'proj')):
    nc = bacc.Bacc()
    wdt = FP8 if QKV_FP8 else BF16
    xt_d = nc.declare_dram_parameter("xt", [C, T], wdt, isOutput=False)
    w_d = nc.declare_dram_parameter("wqkv", [C, 576], wdt, isOutput=False)
    b_d = nc.declare_dram_parameter("bqkv", [128, 5], F32, isOutput=False)
    wp_d = nc.declare_dram_parameter("wp", [192, C], BF16, isOutput=False)
    y_d = nc.declare_dram_parameter("y", [T, C], F32, isOutput=True)
    if QKV_FP8:
        xv0_d = nc.declare_dram_parameter("xv0", [C, 128], BF16,
                                          isOutput=False)
        wv_d = nc.declare_dram_parameter("wv", [C, 192], BF16,
                                         isOutput=False)
        wqk0_d = nc.declare_dram_parameter("wqk0", [C, 384], BF16,
                                           isOutput=False)
    else:
        xv0_d = wv_d = wqk0_d = None
    dram = (xt_d, w_d, b_d, wp_d, y_d, xv0_d, wv_d, wqk0_d)
    with tile.TileContext(nc) as tc:
        with (
            tc.tile_pool(name="persist", bufs=1) as pp,
            tc.tile_pool(name="sb_att", bufs=E_BUFS) as sbE,
            tc.tile_pool(name="sb_n", bufs=N_BUFS) as sbN,
            tc.tile_pool(name="sb_y", bufs=Y_BUFS) as sbY,
            tc.tile_pool(name="psum", bufs=1, space="PSUM") as psp,
        ):
            pools = (pp, sbE, sbN, sbY, psp)
            ident = pp.tile([128, 128], BF16, name="ident")
            make_identity(nc, ident)
            if loop_n > 1:
                assert loop_n % 2 == 1 and loop_n >= 3
                s0 = _alloc_slot(pp, 0)
                s1 = _alloc_slot(pp, 1)
                _build_body(nc, tc, pools, ident, s0, *dram, phases=phases,
                            warmup=WARMUP_MM)
                with tc.For_i(0, (loop_n - 1) // 2, 1):
                    _build_body(nc, tc, pools, ident, s1, *dram,
                                phases=phases)
                    _build_body(nc, tc, pools, ident, s0, *dram,
                                phases=phases)
            else:
                s0 = _alloc_slot(pp, 0)
                _build_body(nc, tc, pools, ident, s0, *dram, phases=phases,
                            warmup=WARMUP_MM)
    nc.compile()
    return nc


def make_in_maps(x, W_attn, b_attn, W_proj):
    """Shard full inputs into the 8 per-core input maps."""
    x = np.asarray(x, np.float32)
    W_attn = np.asarray(W_attn, np.float32)
    b_attn = np.asarray(b_attn, np.float32)
    W_proj = np.asarray(W_proj, np.float32)
    conv = to_fp8 if QKV_FP8 else to_bf16
    xts = [conv(x[b].T) for b in range(B)]
    in_maps = []
    for c in range(NCORES):
        b = c // (NCORES // B)
        heads = [(c % (NCORES // B)) * HPC + j for j in range(HPC)]
        cols, bias = [], []
        for kind, hi in W_ORDER:
            lo = kind * C + heads[hi] * DH
            cols.append(W_attn[:, lo:lo + DH])
            bias.append(b_attn[lo:lo + DH])
        wqkv = np.ascontiguousarray(np.concatenate(cols, axis=1)) * W_SCALE
        bq = np.concatenate(bias + [np.zeros(64, np.float32)]) * W_SCALE
        bq = np.ascontiguousarray(bq.reshape(5, 128).T.astype(np.float32))
        wp = np.concatenate(
            [W_proj[hh * DH:(hh + 1) * DH, :] for hh in heads],
            axis=0) / W_SCALE
        im = {"xt": xts[b], "wqkv": conv(wqkv),
              "bqkv": bq, "wp": to_bf16(wp)}
        if QKV_FP8:
            # bf16 recompute inputs for the first 128 tokens: fp8 qkv error
            # is unaveraged for sharp (early-row) attention
            wv = np.concatenate(
                [W_attn[:, 2 * C + hh * DH:2 * C + (hh + 1) * DH]
                 for hh in heads], axis=1) * W_SCALE
            im["xv0"] = to_bf16(x[b].T[:, 0:128])
            im["wv"] = to_bf16(wv)
            wq = [W_attn[:, heads[hi] * DH:heads[hi] * DH + DH]
                  for hi in range(3)]
            wk = [W_attn[:, C + heads[hi] * DH:C + heads[hi] * DH + DH]
                  for hi in range(3)]
            wqk0 = np.concatenate(wq[:2] + wk[:2] + [wq[2], wk[2]],
                                  axis=1) * W_SCALE
            im["wqk0"] = to_bf16(wqk0)
        in_maps.append(im)
    return in_maps


_module_cache = {}


def kernel(x, W_attn, b_attn, W_proj, b_proj):
    if "nc" not in _module_cache:
        _module_cache["nc"] = build_module()
    nc = _module_cache["nc"]
    in_maps = make_in_maps(x, W_attn, b_attn, W_proj)
    res = run_bass_kernel_spmd(nc, in_maps, core_ids=list(range(NCORES)))
    y = np.zeros((B, T, C), np.float64)
    for c in range(NCORES):
        y[c // (NCORES // B)] += res.results[c]["y"].astype(np.float64)
    y += np.asarray(b_proj, np.float64)
    return y.astype(np.float32)
